# revision 1
# baseline (speedup 1.0000x reference)
"""Trainium2 Bass kernel for nn_DeStationaryCausalAttention.

The reference returns only the LAST query position's output, so the full
L x L attention collapses: per batch we only need

    logits[h, k] = q_eff[h] . K[k, h-slice]      (k over all 2048 keys)
    out          = softmax(logits) @ V  -> @ Wo + bo

with q_eff = tau * q_c / sqrt(32) + delta_last.  Folding q_eff through Wk
gives a per-batch matrix G (16 x 1024) with logits = G @ h^T, and folding
Wv out of the weighted sum gives out from u = softmax(logits) @ h.  The
device therefore only computes logits = h @ G^T and the flash-style
per-chunk stats (m, s, u) over its shard of keys; everything else is tiny
host math.

Sharding: the 4096 (batch, key) rows are split into 8 chunks of 512 keys,
one per NeuronCore (cores 0-3 -> batch 0, cores 4-7 -> batch 1).  Each core
reads its 2 MB h-shard once (memory roofline), transposes it on TensorE for
the D-contraction, computes logits, local softmax stats and the weighted
h-sum u.  Host combines the per-chunk flash stats exactly.
"""

import math

import numpy as np

# Problem shapes (hardcoded per the harness contract).
B, L, D = 2, 2048, 1024
H, HD, KVHD, DKV = 16, 64, 32, 512
NCORES = 8
CHUNK = (B * L) // NCORES       # 512 keys per core
P = 128
KT = CHUNK // P                 # 4 key tiles per core
DT = D // P                     # 8 model-dim tiles

_CACHE = {}


def _fix_sync_waits(nc, maxw=1):
    """Walrus (CoreV3) rejects instructions carrying more than one sync-wait
    command.  Tile's end-of-kernel drain collects one wait per outstanding
    semaphore, so split excess waits onto preceding same-engine NoOps."""
    import concourse.mybir as mybir

    ctr = 0
    for fn in nc.m.functions:
        for blk in fn.blocks:
            new = []
            changed = False
            for inst in blk.instructions:
                si = inst.sync_info
                if si is not None and si.on_wait and len(si.on_wait) > maxw:
                    waits = list(si.on_wait)
                    extra, keep = waits[:-maxw], waits[-maxw:]
                    for i in range(0, len(extra), maxw):
                        nop = mybir.InstNoOp(
                            name=f"waitfix-{ctr}", ins=[], outs=[])
                        ctr += 1
                        nop.engine = inst.engine
                        nop.sync_info = mybir.SyncInfo(
                            on_wait=extra[i:i + maxw], on_update=[])
                        new.append(nop)
                    si.on_wait = keep
                    changed = True
                new.append(inst)
            if changed:
                blk.instructions = new


def _build_nc():
    from contextlib import ExitStack

    import concourse.bass as bass
    import concourse.tile as tile
    from concourse import mybir
    from concourse.masks import make_identity

    f32 = mybir.dt.float32
    nc = bass.Bass("TRN2", debug=False, num_devices=NCORES)
    h_d = nc.dram_tensor("hc", [CHUNK, D], f32, kind="ExternalInput").ap()
    gt_d = nc.dram_tensor("gt", [D, H], f32, kind="ExternalInput").ap()
    u_d = nc.dram_tensor("u_out", [H, D], f32, kind="ExternalOutput").ap()
    ms_d = nc.dram_tensor("ms_out", [H, 2], f32, kind="ExternalOutput").ap()

    with tile.TileContext(nc) as tc, ExitStack() as ctx:
        consts = ctx.enter_context(tc.tile_pool(name="consts", bufs=1))
        hpool = ctx.enter_context(tc.tile_pool(name="h", bufs=1))
        small = ctx.enter_context(tc.tile_pool(name="small", bufs=1))
        pst = ctx.enter_context(tc.tile_pool(name="pst", bufs=2, space="PSUM"))
        ps1 = ctx.enter_context(tc.tile_pool(name="ps1", bufs=1, space="PSUM"))

        ident = consts.tile([P, P], f32)
        make_identity(nc, ident)
        gt_sb = consts.tile([P, DT, H], f32)
        nc.sync.dma_start(gt_sb[:], gt_d.rearrange("(n p) c -> p n c", p=P))

        # h shard, natural layout: partitions = keys (4 tiles of 128 keys).
        h_sb = []
        for kt in range(KT):
            t = hpool.tile([P, D], f32, tag=f"h{kt}")
            nc.sync.dma_start(t[:], h_d[kt * P:(kt + 1) * P, :])
            h_sb.append(t)

        # h^T (partitions = D within tile): 128x128 TensorE transposes,
        # 4 per PSUM bank, one DVE copy per bank.
        hT = small.tile([P, DT, CHUNK], f32, tag="hT")
        for kt in range(KT):
            for g in range(DT // 4):
                ps_t = pst.tile([P, 4, P], f32, tag="pst")
                for j in range(4):
                    dt = g * 4 + j
                    nc.tensor.transpose(
                        ps_t[:, j, :], h_sb[kt][:, dt * P:(dt + 1) * P], ident)
                nc.vector.tensor_copy(
                    hT[:, g * 4:(g + 1) * 4, kt * P:(kt + 1) * P], ps_t[:])

        # logits[h, k] = sum_D gt[D, h] * hT[D, k], accumulated over D tiles.
        ps_lg = ps1.tile([H, CHUNK], f32, tag="lg")
        for dt in range(DT):
            nc.tensor.matmul(
                ps_lg[:], gt_sb[:, dt, :], hT[:, dt, :],
                start=(dt == 0), stop=(dt == DT - 1))

        # clip to +-50 (matches reference), then flash stats.
        lg_sb = small.tile([H, CHUNK], f32, tag="lg_sb")
        nc.vector.tensor_scalar(
            lg_sb[:], ps_lg[:], 50.0, -50.0,
            op0=mybir.AluOpType.min, op1=mybir.AluOpType.max)

        ms_sb = small.tile([H, 2], f32, tag="ms")
        nc.vector.reduce_max(
            ms_sb[:, 0:1], lg_sb[:], axis=mybir.AxisListType.X, negate=True)
        p_sb = small.tile([H, CHUNK], f32, tag="p")
        nc.scalar.activation(
            p_sb[:], lg_sb[:], mybir.ActivationFunctionType.Exp,
            bias=ms_sb[:, 0:1], scale=1.0, accum_out=ms_sb[:, 1:2])
        nc.sync.dma_start(ms_d[:], ms_sb[:])

        # p^T (partitions = keys) for the weighted-sum matmul.
        ps_pt = ps1.tile([P, KT, H], f32, tag="pt")
        for kt in range(KT):
            nc.tensor.transpose(
                ps_pt[:, kt, :], p_sb[:, kt * P:(kt + 1) * P], ident[:H, :H])
        pt_sb = small.tile([P, KT, H], f32, tag="pt_sb")
        nc.vector.tensor_copy(pt_sb[:], ps_pt[:])

        # u[h, :] = sum_k p[h, k] * h[k, :]
        ps_u = ps1.tile([H, D], f32, tag="u")
        for nh in range(D // 512):
            for kt in range(KT):
                nc.tensor.matmul(
                    ps_u[:, nh * 512:(nh + 1) * 512],
                    pt_sb[:, kt, :],
                    h_sb[kt][:, nh * 512:(nh + 1) * 512],
                    start=(kt == 0), stop=(kt == KT - 1))
        u_sb = small.tile([H, D], f32, tag="u_sb")
        nc.vector.tensor_copy(u_sb[:], ps_u[:])
        nc.sync.dma_start(u_d[:], u_sb[:])

    _fix_sync_waits(nc)
    return nc


def _get_nc():
    if "nc" not in _CACHE:
        _CACHE["nc"] = _build_nc()
    return _CACHE["nc"]


def _gelu_exact(x):
    # erf-based GELU, matches jax.nn.gelu(approximate=False).
    from math import erf
    v = np.vectorize(erf, otypes=[np.float64])
    return 0.5 * x * (1.0 + v(x / math.sqrt(2.0)))


def kernel(h, pre_norm_mu, pre_norm_sigma, Wq, Wk, Wv, Wo, bo,
           tau_w1, tau_b1, tau_w2, tau_b2, del_w1, del_b1, del_w2, del_b2):
    from concourse.bass_utils import run_bass_kernel_spmd

    h = np.asarray(h, np.float32)
    f8 = np.float64

    # --- tiny host math for the last position -------------------------------
    h_last = h[:, -1, :].astype(f8)                                   # (B, D)
    sig_mean = np.clip(
        np.asarray(pre_norm_sigma, f8)[:, -1, :].mean(-1, keepdims=True),
        1e-6, None)
    mu_mean = np.asarray(pre_norm_mu, f8)[:, -1, :].mean(-1, keepdims=True)

    tau = np.exp(np.clip(
        _gelu_exact(np.concatenate([sig_mean, h_last], -1)
                    @ np.asarray(tau_w1, f8) + np.asarray(tau_b1, f8))
        @ np.asarray(tau_w2, f8) + np.asarray(tau_b2, f8), -3.0, 3.0))
    delta = np.clip(
        _gelu_exact(np.concatenate([mu_mean, h_last], -1)
                    @ np.asarray(del_w1, f8) + np.asarray(del_b1, f8))
        @ np.asarray(del_w2, f8) + np.asarray(del_b2, f8), -5.0, 5.0)

    q = h_last @ np.asarray(Wq, f8)                                   # (B, D)
    qc = q.reshape(B, H, HD)[:, :, :KVHD]                             # (B,H,32)
    q_eff = (tau.reshape(B, 1, 1) * qc / math.sqrt(KVHD)
             + delta.reshape(B, H, KVHD))
    Wk_r = np.asarray(Wk, f8).reshape(D, H, KVHD)
    G = np.einsum('bhd,Dhd->bhD', q_eff, Wk_r)                        # (B,H,D)
    Gt = np.ascontiguousarray(G.transpose(0, 2, 1)).astype(np.float32)

    # --- device: logits + flash stats + weighted h-sums per key shard -------
    in_maps = []
    for c in range(NCORES):
        b, ck = divmod(c, NCORES // B)
        in_maps.append({
            "hc": np.ascontiguousarray(h[b, ck * CHUNK:(ck + 1) * CHUNK, :]),
            "gt": Gt[b],
        })
    res = run_bass_kernel_spmd(_get_nc(), in_maps, core_ids=list(range(NCORES)))
    results = res.results

    # --- exact flash combine + output projection ----------------------------
    nshard = NCORES // B
    out = np.zeros((B, D), np.float32)
    Wv_r = np.asarray(Wv, f8).reshape(D, H, KVHD)
    for b in range(B):
        ms = [results[b * nshard + ck]["ms_out"] for ck in range(nshard)]
        us = [results[b * nshard + ck]["u_out"] for ck in range(nshard)]
        m_c = np.stack([-m[:, 0] for m in ms]).astype(f8)             # (4, H)
        s_c = np.stack([m[:, 1] for m in ms]).astype(f8)
        u_c = np.stack(us).astype(f8)                                 # (4,H,D)
        M = m_c.max(0)
        w = np.exp(m_c - M)
        S = (s_c * w).sum(0)                                          # (H,)
        U = (u_c * w[:, :, None]).sum(0)                              # (H, D)
        un = U / S[:, None]
        att = np.einsum('hD,Dhd->hd', un, Wv_r)                       # (H, 32)
        out[b] = (att.reshape(DKV) @ np.asarray(Wo, f8)
                  + np.asarray(bo, f8)).astype(np.float32)
    return out
